# revision 20
# baseline (speedup 1.0000x reference)
"""BCE + connectivity loss kernel for Trainium2 (8 NeuronCores, data parallel).

Math (matches the jax reference):
  bce  = mean(-(t * clog(p) + (1-t) * clog(1-p)))   with clog = clip(log, -100)
  pen  = mean_b(num_components(preds[b] != 0) - 1)
  out  = bce + pen

The harness inputs are uniform in [1e-4, 1-1e-4]:
  * log(p), log(1-p) are in (-9.3, 0), so the -100 clamp never binds;
  * preds != 0 is all-True, so every sample has exactly 1 component and
    pen == 0.  (A host-side numpy fallback handles the p==0 case anyway.)

Device computation per core (8 samples = 2,097,152 elems viewed [128,16384]),
using  t*a + (1-t)*b = t*a - (t-1)*b  with a = ln(p), b = ln(1-p):
  ACT:  a_c = ln(p_c), b_c = ln(1-p_c)          per 2048-col tile
  DVE:  S_ta[c]  = sum((t+0)*a)                 (STT, fused mul+reduce)
  POOL: S_t1b[c] = sum((t-1)*b)                 (STT with scalar=-1)
  host: bce = -(sum S_ta - sum S_t1b) / N       (+ 0 penalty)

Performance model (from trace analysis; span = walrus boot ~5.2us ->
first DMA, ~44-47us HBM-bound stream, compute trail, ~8us walrus
epilogue):
  * HBM pair limit: one stack (~716-800 GB/s measured) serves 2 cores,
    so the 33.5 MB per pair needs ~44-47us of streaming no matter what.
    The steady state must stay DMA-paced, never compute-paced; per-run,
    one or two cores randomly get ~5-8us less HBM service (boot
    instruction loads or stream arbitration) and become the max core.
  * Replacing the framework's 5-engine const-memset barrier with a
    single Pool->ACT semaphore lets SP start streaming right after its
    walrus boot (~5.2us vs ~8.3us).
  * DVE f32 2-source STT runs 1.34 ns/col when the two source byte
    addresses differ by a multiple of 32 KiB, 0.96 ns/col otherwise;
    the "bankpad" SBUF tensor de-aliases t_b vs a_b/b_b, which takes
    both STT streams (4.6us/2048-chunk) under the DMA pace.  Offloading
    one stream to Pool was tried and is far WORSE: Pool TT f32 runs
    4.5-6.5us/2048 and its SBUF port contention slows concurrent DVE
    STTs to ~6.5us.
  * The walrus epilogue (fixed ~53 sem-resets/engine sweep + a ~2us
    final-DMA-completion wait) is ~8us and does not scale with the
    kernel's semaphore count -- but fewer semaphores cost nothing, so
    all loads share one cumulative sem on the single SP HWDGE queue
    (FIFO completion), with thresholds 16*(2k+1)/16*(2k+2) for p_k/t_k.
  * Tiles taper at both ends (512,1536,2048x6,1536,512): the small
    first tile + a dummy 1-col Ln (pre-loads the ACT table during p0's
    flight) start ACT ~3.4us earlier; the small last tile cuts the
    post-last-byte trail to ACT-b(512)+2xSTT(512) ~= 1.5us.
  * A soft-start throttle holds bulk DMA issue until the dummy Ln
    completes (~7us): near-free for winners (they have end slack) and
    it protects boot-starved cores a bit.
  * PE is fully stripped from the kernel (no drains/barriers); walrus
    still boots it, but the strip is free.
"""

import numpy as np

# ---------------------------------------------------------------- constants
B, H, W = 64, 512, 512
N_CORES = 8
B_PER_CORE = B // N_CORES            # 8 samples per core
P = 128                              # SBUF partitions
ELEMS_PER_CORE = B_PER_CORE * H * W  # 2_097_152
FREE = ELEMS_PER_CORE // P           # 16384
N_TOTAL = B * H * W

DMA_TILES = (512, 1536) + (2048,) * 6 + (1536, 512)
CHUNK = 2048
AB_BUFS = 4

_CACHE = {}


def _ensure_paths():
    import sys

    for p in ("/root/.axon_site/_ro/trn_rl_repo", "/opt/trn_rl_repo"):
        try:
            import concourse  # noqa: F401

            return
        except ImportError:
            if p not in sys.path:
                sys.path.insert(0, p)
    import concourse  # noqa: F401


def _chunks_of(tile_sizes, chunk=CHUNK, last_split=512):
    """[(tile_idx, _, col_off_in_tile, size), ...] splitting tiles <=chunk.

    The LAST tile is additionally split into `last_split`-col compute
    chunks: the end-of-stream trail is ACT(b) + STT(b) of the final
    chunk, so a small final chunk shortens the critical tail without
    adding DMA completions."""
    out = []
    off = 0
    n = len(tile_sizes)
    for k, fs in enumerate(tile_sizes):
        step = min(chunk, last_split if (k == n - 1 and last_split) else chunk)
        o = 0
        while o < fs:
            c = min(step, fs - o)
            out.append((k, off + o, o, c))
            o += c
        off += fs
    return out


def _build(
    tile_sizes=DMA_TILES,
    chunk=CHUNK,
    ab_bufs=AB_BUFS,
    single_load_sem=True,
    light_const_barrier=True,
    strip_pe=True,
    pool_b=False,
    drop_exit_barrier=True,
):
    assert sum(tile_sizes) == FREE
    _ensure_paths()
    import concourse.bacc as bacc
    import concourse.mybir as mybir

    f32 = mybir.dt.float32
    n = len(tile_sizes)
    offs = [sum(tile_sizes[:i]) for i in range(n)]
    chunks = _chunks_of(tile_sizes, chunk)
    m = len(chunks)
    nc = bacc.Bacc("TRN2", target_bir_lowering=False)
    preds = nc.dram_tensor("preds", [P, FREE], f32, kind="ExternalInput")
    targets = nc.dram_tensor("targets", [P, FREE], f32, kind="ExternalInput")
    # acc col c: [0..m) sum_ta ; [m..2m) sum_(t-1)b
    out_acc = nc.dram_tensor("acc", [P, 2 * m], f32, kind="ExternalOutput")
    mult = mybir.AluOpType.mult
    add = mybir.AluOpType.add
    Ln = mybir.ActivationFunctionType.Ln

    p_b = [nc.alloc_sbuf_tensor(f"pb{i}", [P, fs], f32) for i, fs in enumerate(tile_sizes)]
    t_b = [nc.alloc_sbuf_tensor(f"tb{i}", [P, fs], f32) for i, fs in enumerate(tile_sizes)]
    # 2 KiB/partition pad so (t_b[k] - a_b[j]) byte deltas avoid multiples
    # of 32 KiB: DVE's 2-source f32 STT drops from ~1.34 ns/col to
    # ~0.96 ns/col when the two source addresses don't alias (measured).
    nc.alloc_sbuf_tensor("bankpad", [P, 512], f32)
    a_b = [nc.alloc_sbuf_tensor(f"ab{k}", [P, chunk], f32) for k in range(ab_bufs)]
    b_b = [nc.alloc_sbuf_tensor(f"bb{k}", [P, chunk], f32) for k in range(ab_bufs)]
    acc = nc.alloc_sbuf_tensor("accs", [P, 2 * m], f32)

    if single_load_sem:
        s_load = nc.alloc_semaphore("s_load")

        def p_ready(eng, k):
            eng.wait_ge(s_load, 16 * (2 * k + 1))

        def t_ready(eng, k):
            eng.wait_ge(s_load, 16 * (2 * k + 2))

        def load_inc(bi):
            return bi.then_inc(s_load, 16)
    else:
        s_p = [nc.alloc_semaphore(f"s_p{i}") for i in range(n)]
        s_t = [nc.alloc_semaphore(f"s_t{i}") for i in range(n)]

        def p_ready(eng, k):
            eng.wait_ge(s_p[k], 16)

        def t_ready(eng, k):
            eng.wait_ge(s_t[k], 16)

        def load_inc(bi, _c=[0]):
            i = _c[0] // 2
            sem = s_p[i] if _c[0] % 2 == 0 else s_t[i]
            _c[0] += 1
            return bi.then_inc(sem, 16)

    s_act = nc.alloc_semaphore("s_act")
    s_dve = nc.alloc_semaphore("s_dve")
    s_pool = nc.alloc_semaphore("s_pool") if pool_b else None
    s_const = nc.alloc_semaphore("s_const") if light_const_barrier else None

    if light_const_barrier:
        # Replace the framework's 5-engine const-memset barrier with a
        # single Pool->ACT semaphore: only ACT reads the const APs (the
        # activation bias), DVE/Pool STT scalars are immediates and the
        # SP DMAs touch nothing Pool initializes.  This also unblocks the
        # whole DMA stream: SP no longer rendezvouses before issuing.
        main_blk = next(b for b in nc.m.functions[0].blocks if b.name == "main")
        il = main_blk.instructions
        il[:] = [
            ins
            for ins in il
            if not (
                isinstance(ins, mybir.InstDrain)
                or (
                    isinstance(ins, mybir.InstEventSemaphore)
                    and getattr(ins, "name", "").startswith("barrier_")
                )
            )
        ]
        nc.gpsimd.sem_inc(s_const, 1)

    # prefetch tile-0 loads: emitted in `main` (pre-Block), so SP fires
    # them immediately after its walrus boot, before anything else.
    f0 = tile_sizes[0]
    load_inc(nc.sync.dma_start(out=p_b[0][:, 0:f0], in_=preds[:, 0:f0]))
    load_inc(nc.sync.dma_start(out=t_b[0][:, 0:f0], in_=targets[:, 0:f0]))

    # Work split (pool_b=True):
    #   chunk c < m-1 ("pool chunks"):
    #     ACT:  a = ln(p)                         -> a_b
    #     ACT:  b = ln(1-p), accum -> acc[m+c]    -> b_b   (sum(b) free on ACT)
    #     POOL: d = a - b (TT, in-place over a_b)          (STT is illegal on
    #                                                       Pool; TT is legal)
    #     DVE:  STT (t+0)*d, accum -> acc[c]               (sum(t*d))
    #     contribution: sum(b) + sum(t*(a-b)) = sum(t*a + (1-t)*b)
    #   chunk m-1 (trail): all-DVE two-STT form, acc[c]=sum(t*a),
    #     acc[m+c]=sum((t-1)*b) -- avoids Pool's Q7 launch on the critical
    #     trail.  Host combine flips the sign of the last B column.
    pool_set = set(range(m - 1)) if pool_b else set()
    pool_upto = [len([q for q in range(cc + 1) if q in pool_set]) for cc in range(m)]
    pool_total = len(pool_set)
    # s_dve value after DVE finishes its work for chunk c
    dve_after = []
    dv = 0
    for c in range(m):
        dv += 1 if c in pool_set else 2
        dve_after.append(dv)
    dve_total = dv
    signs_b = [1.0 if c in pool_set else -1.0 for c in range(m)]

    with nc.Block(no_gpsimd_drain=True) as block:

        @block.sync
        def _(sync):
            if light_const_barrier:
                # Soft start: hold the bulk stream until ACT's dummy table
                # load is done (~7us).  8 cores launching full-rate streams
                # at ~5.2us starve the slowest core's walrus boot (its PE
                # init + queue loads compete for HBM), producing random
                # +5-8us boot stragglers -- and the max core is the metric.
                sync.wait_ge(s_const, 2)
            for i, fs in enumerate(tile_sizes):
                if i == 0:
                    continue
                sl = slice(offs[i], offs[i] + fs)
                load_inc(sync.dma_start(out=p_b[i][:, 0:fs], in_=preds[:, sl]))
                load_inc(sync.dma_start(out=t_b[i][:, 0:fs], in_=targets[:, sl]))
            sync.wait_ge(s_dve, dve_total)
            # final store: nothing waits on it; its HBM receipt hides
            # behind the fixed walrus epilogue.  (walrus requires every
            # DMA to carry a sem update, so piggyback on the load sem.)
            store = sync.dma_start(out=out_acc[:, :], in_=acc[:, :])
            if single_load_sem:
                store.then_inc(s_load, 16)
            else:
                store.then_inc(s_act, 16)

        @block.scalar
        def _(scalar):
            if light_const_barrier:
                scalar.wait_ge(s_const, 1)
            # 1-col dummy Ln: pulls the ~1.3us ACT table load off the
            # first-tile critical path (it runs while p0 is in flight).
            # Its s_const inc releases SP's bulk-stream throttle.
            dummy = scalar.activation(
                out=a_b[0][:, 0:1], in_=a_b[0][:, 0:1], func=Ln
            )
            if light_const_barrier:
                dummy.then_inc(s_const, 1)
            seen_tile = -1
            for c, (k, _, o, fs) in enumerate(chunks):
                if k != seen_tile:
                    p_ready(scalar, k)
                    seen_tile = k
                if c >= ab_bufs:
                    # a_b[c % ab_bufs] free once DVE consumed chunk c-ab_bufs
                    scalar.wait_ge(s_dve, dve_after[c - ab_bufs])
                scalar.activation(
                    out=a_b[c % ab_bufs][:, 0:fs],
                    in_=p_b[k][:, o : o + fs],
                    func=Ln,
                ).then_inc(s_act, 1)
                if c >= ab_bufs:
                    # b_b[c % ab_bufs] freed by its consumer
                    cc = c - ab_bufs
                    if cc in pool_set:
                        scalar.wait_ge(s_pool, pool_upto[cc])
                    else:
                        scalar.wait_ge(s_dve, dve_after[cc])
                bact = scalar.activation(
                    out=b_b[c % ab_bufs][:, 0:fs],
                    in_=p_b[k][:, o : o + fs],
                    func=Ln,
                    bias=1.0,
                    scale=-1.0,
                    accum_out=(acc[:, m + c : m + c + 1] if c in pool_set else None),
                )
                bact.then_inc(s_act, 1)

        @block.vector
        def _(vector):
            seen_tile = -1
            for c, (k, _, o, fs) in enumerate(chunks):
                if k != seen_tile:
                    t_ready(vector, k)
                    seen_tile = k
                if c in pool_set:
                    # d = a-b ready once Pool's TT for c is done
                    vector.wait_ge(s_pool, pool_upto[c])
                    d_t = a_b[c % ab_bufs][:, 0:fs]
                    vector.scalar_tensor_tensor(
                        out=d_t,
                        in0=t_b[k][:, o : o + fs],
                        scalar=0.0,
                        in1=d_t,
                        op0=add,
                        op1=mult,
                        accum_out=acc[:, c : c + 1],
                    ).then_inc(s_dve, 1)
                else:
                    vector.wait_ge(s_act, 2 * c + 1)
                    a_t = a_b[c % ab_bufs][:, 0:fs]
                    vector.scalar_tensor_tensor(
                        out=a_t,
                        in0=t_b[k][:, o : o + fs],
                        scalar=0.0,
                        in1=a_t,
                        op0=add,
                        op1=mult,
                        accum_out=acc[:, c : c + 1],
                    ).then_inc(s_dve, 1)
                    vector.wait_ge(s_act, 2 * c + 2)
                    b_t = b_b[c % ab_bufs][:, 0:fs]
                    vector.scalar_tensor_tensor(
                        out=b_t,
                        in0=t_b[k][:, o : o + fs],
                        scalar=-1.0,
                        in1=b_t,
                        op0=add,
                        op1=mult,
                        accum_out=acc[:, m + c : m + c + 1],
                    ).then_inc(s_dve, 1)

        if pool_total:

            @block.gpsimd
            def _(gpsimd):
                for c, (k, _, o, fs) in enumerate(chunks):
                    if c not in pool_set:
                        continue
                    gpsimd.wait_ge(s_act, 2 * c + 2)
                    gpsimd.tensor_tensor(
                        out=a_b[c % ab_bufs][:, 0:fs],
                        in0=a_b[c % ab_bufs][:, 0:fs],
                        in1=b_b[c % ab_bufs][:, 0:fs],
                        op=mybir.AluOpType.subtract,
                    ).then_inc(s_pool, 1)

    import concourse.mybir as _mybir

    if drop_exit_barrier:
        # The Block-exit sem-only barrier (aeb_*) is redundant: walrus's
        # epilogue performs its own rendezvous right after.
        for blk in nc.m.functions[0].blocks:
            if blk.name == "main" or blk.name.endswith("_end"):
                blk.instructions = [
                    ins
                    for ins in blk.instructions
                    if not (
                        isinstance(ins, _mybir.InstEventSemaphore)
                        and getattr(ins, "name", "").startswith("aeb_")
                    )
                ]
    if strip_pe:
        # PE executes nothing; remove its drains so the kernel's BIR has
        # zero PE instructions.
        for blk in nc.m.functions[0].blocks:
            blk.instructions = [
                ins
                for ins in blk.instructions
                if getattr(ins, "engine", None) != _mybir.EngineType.PE
            ]
    nc.compile()
    nc._combine_signs = np.asarray(signs_b, dtype=np.float64)
    nc._combine_m = m
    return nc


N_CHUNKS = len(_chunks_of(DMA_TILES, CHUNK))


def _get_nc():
    if "nc" not in _CACHE:
        _CACHE["nc"] = _build()
    return _CACHE["nc"]


def bass_exec(preds, targets, nc=None):
    """Run the per-core Bass kernel on all 8 cores; returns results list."""
    _ensure_paths()
    from concourse.bass_utils import run_bass_kernel_spmd

    if nc is None:
        nc = _get_nc()
    in_maps = []
    for c in range(N_CORES):
        sl = slice(c * B_PER_CORE, (c + 1) * B_PER_CORE)
        in_maps.append(
            {
                "preds": np.ascontiguousarray(preds[sl]).reshape(P, FREE),
                "targets": np.ascontiguousarray(targets[sl]).reshape(P, FREE),
            }
        )
    return run_bass_kernel_spmd(nc, in_maps, core_ids=list(range(N_CORES)))


def _combine(results, m=None, signs_b=None):
    if m is None:
        nc = _CACHE.get("nc")
        m = nc._combine_m if nc is not None else N_CHUNKS
        if signs_b is None and nc is not None:
            signs_b = nc._combine_signs
    if signs_b is None:
        signs_b = -np.ones(m)
    total = 0.0
    for core_out in results:
        a = np.asarray(core_out["acc"], dtype=np.float64)
        total += a[:, :m].sum() + (a[:, m:] * signs_b).sum()
    return -total / N_TOTAL


def _count_components(mask):
    """Connected-component count, 4-connectivity (reference-equivalent)."""
    try:
        from scipy import ndimage

        return float(ndimage.label(mask)[1])
    except ImportError:
        pass
    return _count_components_np(mask)


def _count_components_np(mask):
    """Pure-numpy fallback: min-label propagation with pointer jumping."""
    Hm, Wm = mask.shape
    N = Hm * Wm
    idx = np.arange(N, dtype=np.int64).reshape(Hm, Wm)
    BIG = np.int64(N)
    lab = np.where(mask, idx, BIG)
    while True:
        up = np.concatenate([lab[1:], np.full((1, Wm), BIG, lab.dtype)], 0)
        down = np.concatenate([np.full((1, Wm), BIG, lab.dtype), lab[:-1]], 0)
        left = np.concatenate([lab[:, 1:], np.full((Hm, 1), BIG, lab.dtype)], 1)
        right = np.concatenate([np.full((Hm, 1), BIG, lab.dtype), lab[:, :-1]], 1)
        nm = np.minimum(np.minimum(up, down), np.minimum(left, right))
        new = np.where(mask, np.minimum(lab, nm), BIG)
        for _ in range(2):  # pointer jumping
            flat = new.reshape(-1)
            valid = flat < N
            safe = np.where(valid, flat, 0)
            flat = np.where(valid, flat[safe], BIG)
            new = flat.reshape(Hm, Wm)
        if np.array_equal(new, lab):
            break
        lab = new
    return float(np.sum(mask & (lab == idx)))


def kernel(preds, targets):
    preds = np.asarray(preds, dtype=np.float32)
    targets = np.asarray(targets, dtype=np.float32)
    assert preds.shape == (B, H, W) and targets.shape == (B, H, W)

    res = bass_exec(preds, targets)
    bce = _combine(res.results)
    if not np.isfinite(bce):
        # a wedged/just-recovered device can return garbage once; one
        # clean re-execution flushes it
        res = bass_exec(preds, targets)
        bce = _combine(res.results)

    # connectivity penalty: 0 unless preds contains exact zeros
    if np.any(preds == 0.0):
        counts = [_count_components(preds[b] != 0.0) for b in range(B)]
        penalty = float(np.mean(np.asarray(counts) - 1.0))
    else:
        penalty = 0.0

    return np.float32(bce + penalty)


# revision 23
# speedup vs baseline: 1.0713x; 1.0713x over previous
"""BCE + connectivity loss kernel for Trainium2 (8 NeuronCores, data parallel).

Math (matches the jax reference):
  bce  = mean(-(t * clog(p) + (1-t) * clog(1-p)))   with clog = clip(log, -100)
  pen  = mean_b(num_components(preds[b] != 0) - 1)
  out  = bce + pen

The harness inputs are uniform in [1e-4, 1-1e-4]:
  * log(p), log(1-p) are in (-9.3, 0), so the -100 clamp never binds;
  * preds != 0 is all-True, so every sample has exactly 1 component and
    pen == 0.  (A host-side numpy fallback handles the p==0 case anyway.)

Device computation per core (8 samples = 2,097,152 elems viewed [128,16384]),
using  t*a + (1-t)*b = t*a - (t-1)*b  with a = ln(p), b = ln(1-p):
  ACT:  a_c = ln(p_c), b_c = ln(1-p_c)          per 2048-col tile
  DVE:  S_ta[c]  = sum((t+0)*a)                 (STT, fused mul+reduce)
  POOL: S_t1b[c] = sum((t-1)*b)                 (STT with scalar=-1)
  host: bce = -(sum S_ta - sum S_t1b) / N       (+ 0 penalty)

Performance model (from trace analysis; span = walrus boot ~5.2us ->
first DMA, ~44-47us HBM-bound stream, compute trail, ~8us walrus
epilogue):
  * HBM pair limit: one stack (~716-800 GB/s measured) serves 2 cores,
    so the 33.5 MB per pair needs ~44-47us of streaming no matter what.
    The steady state must stay DMA-paced, never compute-paced; per-run,
    one or two cores randomly get ~5-8us less HBM service (boot
    instruction loads or stream arbitration) and become the max core.
  * Replacing the framework's 5-engine const-memset barrier with a
    single Pool->ACT semaphore lets SP start streaming right after its
    walrus boot (~5.2us vs ~8.3us).
  * DVE f32 2-source STT runs 1.34 ns/col when the two source byte
    addresses differ by a multiple of 32 KiB, 0.96 ns/col otherwise;
    the "bankpad" SBUF tensor de-aliases t_b vs a_b/b_b, which takes
    both STT streams (4.6us/2048-chunk) under the DMA pace.  Offloading
    one stream to Pool was tried and is far WORSE: Pool TT f32 runs
    4.5-6.5us/2048 and its SBUF port contention slows concurrent DVE
    STTs to ~6.5us.
  * The walrus epilogue (fixed ~53 sem-resets/engine sweep + a ~2us
    final-DMA-completion wait) is ~8us and does not scale with the
    kernel's semaphore count -- but fewer semaphores cost nothing, so
    all loads share one cumulative sem on the single SP HWDGE queue
    (FIFO completion), with thresholds 16*(2k+1)/16*(2k+2) for p_k/t_k.
  * Tiles taper at both ends (512,1536,2048x6,1536,512): the small
    first tile + a dummy 1-col Ln (pre-loads the ACT table during p0's
    flight) start ACT ~3.4us earlier; the small last tile cuts the
    post-last-byte trail to ACT-b(512)+2xSTT(512) ~= 1.5us.
  * A soft-start throttle (hold bulk DMA until the dummy Ln finishes)
    was tried and REMOVED: it also delays the straggler's own stream
    start, which hurts exactly when the straggler is stream-limited.
  * PE is fully stripped from the kernel (no drains/barriers); walrus
    still boots it, but the strip is free.
"""

import numpy as np

# ---------------------------------------------------------------- constants
B, H, W = 64, 512, 512
N_CORES = 8
B_PER_CORE = B // N_CORES            # 8 samples per core
P = 128                              # SBUF partitions
ELEMS_PER_CORE = B_PER_CORE * H * W  # 2_097_152
FREE = ELEMS_PER_CORE // P           # 16384
N_TOTAL = B * H * W

DMA_TILES = (512, 1536) + (2048,) * 6 + (1536, 512)
CHUNK = 2048
AB_BUFS = 4

_CACHE = {}


def _ensure_paths():
    import sys

    for p in ("/root/.axon_site/_ro/trn_rl_repo", "/opt/trn_rl_repo"):
        try:
            import concourse  # noqa: F401

            return
        except ImportError:
            if p not in sys.path:
                sys.path.insert(0, p)
    import concourse  # noqa: F401


def _chunks_of(tile_sizes, chunk=CHUNK, last_split=512):
    """[(tile_idx, _, col_off_in_tile, size), ...] splitting tiles <=chunk.

    The LAST tile is additionally split into `last_split`-col compute
    chunks: the end-of-stream trail is ACT(b) + STT(b) of the final
    chunk, so a small final chunk shortens the critical tail without
    adding DMA completions."""
    out = []
    off = 0
    n = len(tile_sizes)
    for k, fs in enumerate(tile_sizes):
        step = min(chunk, last_split if (k == n - 1 and last_split) else chunk)
        o = 0
        while o < fs:
            c = min(step, fs - o)
            out.append((k, off + o, o, c))
            o += c
        off += fs
    return out


def _build(
    tile_sizes=DMA_TILES,
    chunk=CHUNK,
    ab_bufs=AB_BUFS,
    single_load_sem=True,
    light_const_barrier=True,
    strip_pe=True,
    pool_b=False,
    drop_exit_barrier=True,
):
    assert sum(tile_sizes) == FREE
    _ensure_paths()
    import concourse.bacc as bacc
    import concourse.mybir as mybir

    f32 = mybir.dt.float32
    n = len(tile_sizes)
    offs = [sum(tile_sizes[:i]) for i in range(n)]
    chunks = _chunks_of(tile_sizes, chunk)
    m = len(chunks)
    nc = bacc.Bacc("TRN2", target_bir_lowering=False)
    preds = nc.dram_tensor("preds", [P, FREE], f32, kind="ExternalInput")
    targets = nc.dram_tensor("targets", [P, FREE], f32, kind="ExternalInput")
    # acc col c: [0..m) sum_ta ; [m..2m) sum_(t-1)b
    out_acc = nc.dram_tensor("acc", [P, 2 * m], f32, kind="ExternalOutput")
    mult = mybir.AluOpType.mult
    add = mybir.AluOpType.add
    Ln = mybir.ActivationFunctionType.Ln

    p_b = [nc.alloc_sbuf_tensor(f"pb{i}", [P, fs], f32) for i, fs in enumerate(tile_sizes)]
    t_b = [nc.alloc_sbuf_tensor(f"tb{i}", [P, fs], f32) for i, fs in enumerate(tile_sizes)]
    # 2 KiB/partition pad so (t_b[k] - a_b[j]) byte deltas avoid multiples
    # of 32 KiB: DVE's 2-source f32 STT drops from ~1.34 ns/col to
    # ~0.96 ns/col when the two source addresses don't alias (measured).
    nc.alloc_sbuf_tensor("bankpad", [P, 512], f32)
    a_b = [nc.alloc_sbuf_tensor(f"ab{k}", [P, chunk], f32) for k in range(ab_bufs)]
    b_b = [nc.alloc_sbuf_tensor(f"bb{k}", [P, chunk], f32) for k in range(ab_bufs)]
    acc = nc.alloc_sbuf_tensor("accs", [P, 2 * m], f32)

    if single_load_sem:
        s_load = nc.alloc_semaphore("s_load")

        def p_ready(eng, k):
            eng.wait_ge(s_load, 16 * (2 * k + 1))

        def t_ready(eng, k):
            eng.wait_ge(s_load, 16 * (2 * k + 2))

        def load_inc(bi):
            return bi.then_inc(s_load, 16)
    else:
        s_p = [nc.alloc_semaphore(f"s_p{i}") for i in range(n)]
        s_t = [nc.alloc_semaphore(f"s_t{i}") for i in range(n)]

        def p_ready(eng, k):
            eng.wait_ge(s_p[k], 16)

        def t_ready(eng, k):
            eng.wait_ge(s_t[k], 16)

        def load_inc(bi, _c=[0]):
            i = _c[0] // 2
            sem = s_p[i] if _c[0] % 2 == 0 else s_t[i]
            _c[0] += 1
            return bi.then_inc(sem, 16)

    s_act = nc.alloc_semaphore("s_act")
    s_dve = nc.alloc_semaphore("s_dve")
    s_pool = nc.alloc_semaphore("s_pool") if pool_b else None
    s_const = nc.alloc_semaphore("s_const") if light_const_barrier else None

    if light_const_barrier:
        # Replace the framework's 5-engine const-memset barrier with a
        # single Pool->ACT semaphore: only ACT reads the const APs (the
        # activation bias), DVE/Pool STT scalars are immediates and the
        # SP DMAs touch nothing Pool initializes.  This also unblocks the
        # whole DMA stream: SP no longer rendezvouses before issuing.
        main_blk = next(b for b in nc.m.functions[0].blocks if b.name == "main")
        il = main_blk.instructions
        il[:] = [
            ins
            for ins in il
            if not (
                isinstance(ins, mybir.InstDrain)
                or (
                    isinstance(ins, mybir.InstEventSemaphore)
                    and getattr(ins, "name", "").startswith("barrier_")
                )
            )
        ]
        nc.gpsimd.sem_inc(s_const, 1)

    # prefetch tile-0 loads: emitted in `main` (pre-Block), so SP fires
    # them immediately after its walrus boot, before anything else.
    f0 = tile_sizes[0]
    load_inc(nc.sync.dma_start(out=p_b[0][:, 0:f0], in_=preds[:, 0:f0]))
    load_inc(nc.sync.dma_start(out=t_b[0][:, 0:f0], in_=targets[:, 0:f0]))

    # Work split (pool_b=True):
    #   chunk c < m-1 ("pool chunks"):
    #     ACT:  a = ln(p)                         -> a_b
    #     ACT:  b = ln(1-p), accum -> acc[m+c]    -> b_b   (sum(b) free on ACT)
    #     POOL: d = a - b (TT, in-place over a_b)          (STT is illegal on
    #                                                       Pool; TT is legal)
    #     DVE:  STT (t+0)*d, accum -> acc[c]               (sum(t*d))
    #     contribution: sum(b) + sum(t*(a-b)) = sum(t*a + (1-t)*b)
    #   chunk m-1 (trail): all-DVE two-STT form, acc[c]=sum(t*a),
    #     acc[m+c]=sum((t-1)*b) -- avoids Pool's Q7 launch on the critical
    #     trail.  Host combine flips the sign of the last B column.
    pool_set = set(range(m - 1)) if pool_b else set()
    pool_upto = [len([q for q in range(cc + 1) if q in pool_set]) for cc in range(m)]
    pool_total = len(pool_set)
    # s_dve value after DVE finishes its work for chunk c
    dve_after = []
    dv = 0
    for c in range(m):
        dv += 1 if c in pool_set else 2
        dve_after.append(dv)
    dve_total = dv
    signs_b = [1.0 if c in pool_set else -1.0 for c in range(m)]

    with nc.Block(no_gpsimd_drain=True) as block:

        @block.sync
        def _(sync):
            for i, fs in enumerate(tile_sizes):
                if i == 0:
                    continue
                sl = slice(offs[i], offs[i] + fs)
                load_inc(sync.dma_start(out=p_b[i][:, 0:fs], in_=preds[:, sl]))
                load_inc(sync.dma_start(out=t_b[i][:, 0:fs], in_=targets[:, sl]))
            sync.wait_ge(s_dve, dve_total)
            # final store: nothing waits on it; its HBM receipt hides
            # behind the fixed walrus epilogue.  (walrus requires every
            # DMA to carry a sem update, so piggyback on the load sem.)
            store = sync.dma_start(out=out_acc[:, :], in_=acc[:, :])
            if single_load_sem:
                store.then_inc(s_load, 16)
            else:
                store.then_inc(s_act, 16)

        @block.scalar
        def _(scalar):
            if light_const_barrier:
                scalar.wait_ge(s_const, 1)
            # 1-col dummy Ln: pulls the ~1.3us ACT table load off the
            # first-tile critical path (it runs while p0 is in flight).
            dummy = scalar.activation(
                out=a_b[0][:, 0:1], in_=a_b[0][:, 0:1], func=Ln
            )
            if light_const_barrier:
                dummy.then_inc(s_const, 1)
            seen_tile = -1
            for c, (k, _, o, fs) in enumerate(chunks):
                if k != seen_tile:
                    p_ready(scalar, k)
                    seen_tile = k
                if c >= ab_bufs:
                    # a_b[c % ab_bufs] free once DVE consumed chunk c-ab_bufs
                    scalar.wait_ge(s_dve, dve_after[c - ab_bufs])
                scalar.activation(
                    out=a_b[c % ab_bufs][:, 0:fs],
                    in_=p_b[k][:, o : o + fs],
                    func=Ln,
                ).then_inc(s_act, 1)
                if c >= ab_bufs:
                    # b_b[c % ab_bufs] freed by its consumer
                    cc = c - ab_bufs
                    if cc in pool_set:
                        scalar.wait_ge(s_pool, pool_upto[cc])
                    else:
                        scalar.wait_ge(s_dve, dve_after[cc])
                bact = scalar.activation(
                    out=b_b[c % ab_bufs][:, 0:fs],
                    in_=p_b[k][:, o : o + fs],
                    func=Ln,
                    bias=1.0,
                    scale=-1.0,
                    accum_out=(acc[:, m + c : m + c + 1] if c in pool_set else None),
                )
                bact.then_inc(s_act, 1)

        @block.vector
        def _(vector):
            seen_tile = -1
            for c, (k, _, o, fs) in enumerate(chunks):
                if k != seen_tile:
                    t_ready(vector, k)
                    seen_tile = k
                if c in pool_set:
                    # d = a-b ready once Pool's TT for c is done
                    vector.wait_ge(s_pool, pool_upto[c])
                    d_t = a_b[c % ab_bufs][:, 0:fs]
                    vector.scalar_tensor_tensor(
                        out=d_t,
                        in0=t_b[k][:, o : o + fs],
                        scalar=0.0,
                        in1=d_t,
                        op0=add,
                        op1=mult,
                        accum_out=acc[:, c : c + 1],
                    ).then_inc(s_dve, 1)
                else:
                    vector.wait_ge(s_act, 2 * c + 1)
                    a_t = a_b[c % ab_bufs][:, 0:fs]
                    vector.scalar_tensor_tensor(
                        out=a_t,
                        in0=t_b[k][:, o : o + fs],
                        scalar=0.0,
                        in1=a_t,
                        op0=add,
                        op1=mult,
                        accum_out=acc[:, c : c + 1],
                    ).then_inc(s_dve, 1)
                    vector.wait_ge(s_act, 2 * c + 2)
                    b_t = b_b[c % ab_bufs][:, 0:fs]
                    vector.scalar_tensor_tensor(
                        out=b_t,
                        in0=t_b[k][:, o : o + fs],
                        scalar=-1.0,
                        in1=b_t,
                        op0=add,
                        op1=mult,
                        accum_out=acc[:, m + c : m + c + 1],
                    ).then_inc(s_dve, 1)

        if pool_total:

            @block.gpsimd
            def _(gpsimd):
                for c, (k, _, o, fs) in enumerate(chunks):
                    if c not in pool_set:
                        continue
                    gpsimd.wait_ge(s_act, 2 * c + 2)
                    gpsimd.tensor_tensor(
                        out=a_b[c % ab_bufs][:, 0:fs],
                        in0=a_b[c % ab_bufs][:, 0:fs],
                        in1=b_b[c % ab_bufs][:, 0:fs],
                        op=mybir.AluOpType.subtract,
                    ).then_inc(s_pool, 1)

    import concourse.mybir as _mybir

    if drop_exit_barrier:
        # The Block-exit sem-only barrier (aeb_*) is redundant: walrus's
        # epilogue performs its own rendezvous right after.
        for blk in nc.m.functions[0].blocks:
            if blk.name == "main" or blk.name.endswith("_end"):
                blk.instructions = [
                    ins
                    for ins in blk.instructions
                    if not (
                        isinstance(ins, _mybir.InstEventSemaphore)
                        and getattr(ins, "name", "").startswith("aeb_")
                    )
                ]
    if strip_pe:
        # PE executes nothing; remove its drains so the kernel's BIR has
        # zero PE instructions.
        for blk in nc.m.functions[0].blocks:
            blk.instructions = [
                ins
                for ins in blk.instructions
                if getattr(ins, "engine", None) != _mybir.EngineType.PE
            ]
    nc.compile()
    nc._combine_signs = np.asarray(signs_b, dtype=np.float64)
    nc._combine_m = m
    return nc


N_CHUNKS = len(_chunks_of(DMA_TILES, CHUNK))


def _get_nc():
    if "nc" not in _CACHE:
        _CACHE["nc"] = _build()
    return _CACHE["nc"]


def bass_exec(preds, targets, nc=None):
    """Run the per-core Bass kernel on all 8 cores; returns results list."""
    _ensure_paths()
    from concourse.bass_utils import run_bass_kernel_spmd

    if nc is None:
        nc = _get_nc()
    in_maps = []
    for c in range(N_CORES):
        sl = slice(c * B_PER_CORE, (c + 1) * B_PER_CORE)
        in_maps.append(
            {
                "preds": np.ascontiguousarray(preds[sl]).reshape(P, FREE),
                "targets": np.ascontiguousarray(targets[sl]).reshape(P, FREE),
            }
        )
    return run_bass_kernel_spmd(nc, in_maps, core_ids=list(range(N_CORES)))


def _combine(results, m=None, signs_b=None):
    if m is None:
        nc = _CACHE.get("nc")
        m = nc._combine_m if nc is not None else N_CHUNKS
        if signs_b is None and nc is not None:
            signs_b = nc._combine_signs
    if signs_b is None:
        signs_b = -np.ones(m)
    total = 0.0
    for core_out in results:
        a = np.asarray(core_out["acc"], dtype=np.float64)
        total += a[:, :m].sum() + (a[:, m:] * signs_b).sum()
    return -total / N_TOTAL


def _count_components(mask):
    """Connected-component count, 4-connectivity (reference-equivalent)."""
    try:
        from scipy import ndimage

        return float(ndimage.label(mask)[1])
    except ImportError:
        pass
    return _count_components_np(mask)


def _count_components_np(mask):
    """Pure-numpy fallback: min-label propagation with pointer jumping."""
    Hm, Wm = mask.shape
    N = Hm * Wm
    idx = np.arange(N, dtype=np.int64).reshape(Hm, Wm)
    BIG = np.int64(N)
    lab = np.where(mask, idx, BIG)
    while True:
        up = np.concatenate([lab[1:], np.full((1, Wm), BIG, lab.dtype)], 0)
        down = np.concatenate([np.full((1, Wm), BIG, lab.dtype), lab[:-1]], 0)
        left = np.concatenate([lab[:, 1:], np.full((Hm, 1), BIG, lab.dtype)], 1)
        right = np.concatenate([np.full((Hm, 1), BIG, lab.dtype), lab[:, :-1]], 1)
        nm = np.minimum(np.minimum(up, down), np.minimum(left, right))
        new = np.where(mask, np.minimum(lab, nm), BIG)
        for _ in range(2):  # pointer jumping
            flat = new.reshape(-1)
            valid = flat < N
            safe = np.where(valid, flat, 0)
            flat = np.where(valid, flat[safe], BIG)
            new = flat.reshape(Hm, Wm)
        if np.array_equal(new, lab):
            break
        lab = new
    return float(np.sum(mask & (lab == idx)))


def kernel(preds, targets):
    preds = np.asarray(preds, dtype=np.float32)
    targets = np.asarray(targets, dtype=np.float32)
    assert preds.shape == (B, H, W) and targets.shape == (B, H, W)

    res = bass_exec(preds, targets)
    bce = _combine(res.results)
    if not np.isfinite(bce):
        # a wedged/just-recovered device can return garbage once; one
        # clean re-execution flushes it
        res = bass_exec(preds, targets)
        bce = _combine(res.results)

    # connectivity penalty: 0 unless preds contains exact zeros
    if np.any(preds == 0.0):
        counts = [_count_components(preds[b] != 0.0) for b in range(B)]
        penalty = float(np.mean(np.asarray(counts) - 1.0))
    else:
        penalty = 0.0

    return np.float32(bce + penalty)


# revision 30
# speedup vs baseline: 1.1028x; 1.0294x over previous
"""BCE + connectivity loss kernel for Trainium2 (8 NeuronCores, data parallel).

Math (matches the jax reference):
  bce  = mean(-(t * clog(p) + (1-t) * clog(1-p)))   with clog = clip(log, -100)
  pen  = mean_b(num_components(preds[b] != 0) - 1)
  out  = bce + pen

The harness inputs are uniform in [1e-4, 1-1e-4]:
  * log(p), log(1-p) are in (-9.3, 0), so the -100 clamp never binds;
  * preds != 0 is all-True, so every sample has exactly 1 component and
    pen == 0.  (A host-side numpy fallback handles the p==0 case anyway.)

Device computation per core (8 samples = 2,097,152 elems viewed [128,16384]),
using  t*a + (1-t)*b = t*a - (t-1)*b  with a = ln(p), b = ln(1-p):
  ACT:  a_c = ln(p_c), b_c = ln(1-p_c)          per 2048-col tile
  DVE:  S_ta[c]  = sum((t+0)*a)                 (STT, fused mul+reduce)
  POOL: S_t1b[c] = sum((t-1)*b)                 (STT with scalar=-1)
  host: bce = -(sum S_ta - sum S_t1b) / N       (+ 0 penalty)

Performance model (from trace analysis; span = walrus boot ~5.2us ->
first DMA, ~44-47us HBM-bound stream, compute trail, ~8us walrus
epilogue):
  * HBM pair limit: one stack (~716-800 GB/s measured) serves 2 cores,
    so the 33.5 MB per pair needs ~44-47us of streaming no matter what.
    The steady state must stay DMA-paced, never compute-paced; per-run,
    one or two cores randomly get ~5-8us less HBM service (boot
    instruction loads or stream arbitration) and become the max core.
  * Replacing the framework's 5-engine const-memset barrier with a
    single Pool->ACT semaphore lets SP start streaming right after its
    walrus boot (~5.2us vs ~8.3us).
  * DVE f32 2-source STT runs 1.34 ns/col when the two source byte
    addresses differ by a multiple of 32 KiB, 0.96 ns/col otherwise;
    the "bankpad" SBUF tensor de-aliases t_b vs a_b/b_b, which takes
    both STT streams (4.6us/2048-chunk) under the DMA pace.  Offloading
    one stream to Pool was tried and is far WORSE: Pool TT f32 runs
    4.5-6.5us/2048 and its SBUF port contention slows concurrent DVE
    STTs to ~6.5us.
  * The walrus epilogue (fixed ~53 sem-resets/engine sweep + a ~2us
    final-DMA-completion wait) is ~8us and does not scale with the
    kernel's semaphore count -- but fewer semaphores cost nothing, so
    all loads share one cumulative sem on the single SP HWDGE queue
    (FIFO completion), with thresholds 16*(2k+1)/16*(2k+2) for p_k/t_k.
  * Tiles taper at both ends (512,1536,2048x6,1792,256): the small
    first tile + a dummy 1-col Ln (pre-loads the ACT table during p0's
    flight) start ACT ~3.4us earlier; the small last tile cuts the
    post-last-byte trail (2xSTT on the final chunk) to ~1.2us.  The
    t-tile of the last pair stays LAST in the queue so ACT's work is
    off the post-completion chain.
  * A soft-start throttle (hold bulk DMA until the dummy Ln finishes)
    was tried and REMOVED: it also delays the straggler's own stream
    start, which hurts exactly when the straggler is stream-limited.
  * PE is fully stripped from the kernel (no drains/barriers); walrus
    still boots it, but the strip is free.
"""

import numpy as np

# ---------------------------------------------------------------- constants
B, H, W = 64, 512, 512
N_CORES = 8
B_PER_CORE = B // N_CORES            # 8 samples per core
P = 128                              # SBUF partitions
ELEMS_PER_CORE = B_PER_CORE * H * W  # 2_097_152
FREE = ELEMS_PER_CORE // P           # 16384
N_TOTAL = B * H * W

DMA_TILES = (512, 1536) + (2048,) * 6 + (1792, 256)
CHUNK = 2048
AB_BUFS = 4

_CACHE = {}


def _ensure_paths():
    import sys

    for p in ("/root/.axon_site/_ro/trn_rl_repo", "/opt/trn_rl_repo"):
        try:
            import concourse  # noqa: F401

            return
        except ImportError:
            if p not in sys.path:
                sys.path.insert(0, p)
    import concourse  # noqa: F401


def _chunks_of(tile_sizes, chunk=CHUNK, last_split=512):
    """[(tile_idx, _, col_off_in_tile, size), ...] splitting tiles <=chunk.

    The LAST tile is additionally split into `last_split`-col compute
    chunks: the end-of-stream trail is ACT(b) + STT(b) of the final
    chunk, so a small final chunk shortens the critical tail without
    adding DMA completions."""
    out = []
    off = 0
    n = len(tile_sizes)
    for k, fs in enumerate(tile_sizes):
        step = min(chunk, last_split if (k == n - 1 and last_split) else chunk)
        o = 0
        while o < fs:
            c = min(step, fs - o)
            out.append((k, off + o, o, c))
            o += c
        off += fs
    return out


def _build(
    tile_sizes=DMA_TILES,
    chunk=CHUNK,
    ab_bufs=AB_BUFS,
    single_load_sem=True,
    light_const_barrier=True,
    strip_pe=True,
    pool_b=False,
    drop_exit_barrier=True,
):
    assert sum(tile_sizes) == FREE
    _ensure_paths()
    import concourse.bacc as bacc
    import concourse.mybir as mybir

    f32 = mybir.dt.float32
    n = len(tile_sizes)
    offs = [sum(tile_sizes[:i]) for i in range(n)]
    chunks = _chunks_of(tile_sizes, chunk)
    m = len(chunks)
    nc = bacc.Bacc("TRN2", target_bir_lowering=False)
    preds = nc.dram_tensor("preds", [P, FREE], f32, kind="ExternalInput")
    targets = nc.dram_tensor("targets", [P, FREE], f32, kind="ExternalInput")
    # acc col c: [0..m) sum_ta ; [m..2m) sum_(t-1)b
    out_acc = nc.dram_tensor("acc", [P, 2 * m], f32, kind="ExternalOutput")
    mult = mybir.AluOpType.mult
    add = mybir.AluOpType.add
    Ln = mybir.ActivationFunctionType.Ln

    p_b = [nc.alloc_sbuf_tensor(f"pb{i}", [P, fs], f32) for i, fs in enumerate(tile_sizes)]
    t_b = [nc.alloc_sbuf_tensor(f"tb{i}", [P, fs], f32) for i, fs in enumerate(tile_sizes)]
    # 2 KiB/partition pad so (t_b[k] - a_b[j]) byte deltas avoid multiples
    # of 32 KiB: DVE's 2-source f32 STT drops from ~1.34 ns/col to
    # ~0.96 ns/col when the two source addresses don't alias (measured).
    nc.alloc_sbuf_tensor("bankpad", [P, 512], f32)
    a_b = [nc.alloc_sbuf_tensor(f"ab{k}", [P, chunk], f32) for k in range(ab_bufs)]
    b_b = [nc.alloc_sbuf_tensor(f"bb{k}", [P, chunk], f32) for k in range(ab_bufs)]
    acc = nc.alloc_sbuf_tensor("accs", [P, 2 * m], f32)

    if single_load_sem:
        s_load = nc.alloc_semaphore("s_load")

        def p_ready(eng, k):
            eng.wait_ge(s_load, 16 * (2 * k + 1))

        def t_ready(eng, k):
            eng.wait_ge(s_load, 16 * (2 * k + 2))

        def load_inc(bi):
            return bi.then_inc(s_load, 16)
    else:
        s_p = [nc.alloc_semaphore(f"s_p{i}") for i in range(n)]
        s_t = [nc.alloc_semaphore(f"s_t{i}") for i in range(n)]

        def p_ready(eng, k):
            eng.wait_ge(s_p[k], 16)

        def t_ready(eng, k):
            eng.wait_ge(s_t[k], 16)

        def load_inc(bi, _c=[0]):
            i = _c[0] // 2
            sem = s_p[i] if _c[0] % 2 == 0 else s_t[i]
            _c[0] += 1
            return bi.then_inc(sem, 16)

    s_act = nc.alloc_semaphore("s_act")
    s_dve = nc.alloc_semaphore("s_dve")
    s_pool = nc.alloc_semaphore("s_pool") if pool_b else None
    s_const = nc.alloc_semaphore("s_const") if light_const_barrier else None

    if light_const_barrier:
        # Replace the framework's 5-engine const-memset barrier with a
        # single Pool->ACT semaphore: only ACT reads the const APs (the
        # activation bias), DVE/Pool STT scalars are immediates and the
        # SP DMAs touch nothing Pool initializes.  This also unblocks the
        # whole DMA stream: SP no longer rendezvouses before issuing.
        main_blk = next(b for b in nc.m.functions[0].blocks if b.name == "main")
        il = main_blk.instructions
        il[:] = [
            ins
            for ins in il
            if not (
                isinstance(ins, mybir.InstDrain)
                or (
                    isinstance(ins, mybir.InstEventSemaphore)
                    and getattr(ins, "name", "").startswith("barrier_")
                )
            )
        ]
        nc.gpsimd.sem_inc(s_const, 1)

    # prefetch tile-0 loads: emitted in `main` (pre-Block), so SP fires
    # them immediately after its walrus boot, before anything else.
    f0 = tile_sizes[0]
    load_inc(nc.sync.dma_start(out=p_b[0][:, 0:f0], in_=preds[:, 0:f0]))
    load_inc(nc.sync.dma_start(out=t_b[0][:, 0:f0], in_=targets[:, 0:f0]))

    # Work split (pool_b=True):
    #   chunk c < m-1 ("pool chunks"):
    #     ACT:  a = ln(p)                         -> a_b
    #     ACT:  b = ln(1-p), accum -> acc[m+c]    -> b_b   (sum(b) free on ACT)
    #     POOL: d = a - b (TT, in-place over a_b)          (STT is illegal on
    #                                                       Pool; TT is legal)
    #     DVE:  STT (t+0)*d, accum -> acc[c]               (sum(t*d))
    #     contribution: sum(b) + sum(t*(a-b)) = sum(t*a + (1-t)*b)
    #   chunk m-1 (trail): all-DVE two-STT form, acc[c]=sum(t*a),
    #     acc[m+c]=sum((t-1)*b) -- avoids Pool's Q7 launch on the critical
    #     trail.  Host combine flips the sign of the last B column.
    pool_set = set(range(m - 1)) if pool_b else set()
    pool_upto = [len([q for q in range(cc + 1) if q in pool_set]) for cc in range(m)]
    pool_total = len(pool_set)
    # s_dve value after DVE finishes its work for chunk c
    dve_after = []
    dv = 0
    for c in range(m):
        dv += 1 if c in pool_set else 2
        dve_after.append(dv)
    dve_total = dv
    # acc column map.  Without pool_b, the LAST chunk's two columns sit
    # at the END (2m-2, 2m-1) so the store can go out in two pieces:
    # the bulk right after chunk m-2 (its ~2us HBM write receipt hides
    # behind the last chunk's STTs) and a tiny 2-column store at the
    # very end -- the only receipt left on the critical tail.
    if pool_b:
        acol = list(range(m))
        bcol = [m + c for c in range(m)]
        split_store = False
    else:
        acol = list(range(m - 1)) + [2 * m - 2]
        bcol = [m - 1 + c for c in range(m - 1)] + [2 * m - 1]
        # A split store (bulk columns early, 2-col store at the end) was
        # tried to hide the ~2us HBM write receipt: it raced the DVE
        # read-accumulator on cold first executions (finite garbage on
        # the run the harness grades).  Not worth ~0.8us.
        split_store = False
    signs_full = np.zeros(2 * m)
    for c in range(m):
        signs_full[acol[c]] = 1.0
        signs_full[bcol[c]] = 1.0 if c in pool_set else -1.0

    with nc.Block(no_gpsimd_drain=True) as block:

        @block.sync
        def _(sync):
            for i, fs in enumerate(tile_sizes):
                if i == 0:
                    continue
                sl = slice(offs[i], offs[i] + fs)
                load_inc(sync.dma_start(out=p_b[i][:, 0:fs], in_=preds[:, sl]))
                load_inc(sync.dma_start(out=t_b[i][:, 0:fs], in_=targets[:, sl]))
            # (walrus requires every DMA to carry a sem update, so the
            # stores piggyback on a sem nothing waits at these values.)
            store_sem = s_load if single_load_sem else s_act
            if split_store:
                sync.wait_ge(s_dve, dve_after[m - 2])
                sync.dma_start(
                    out=out_acc[:, 0 : 2 * m - 2], in_=acc[:, 0 : 2 * m - 2]
                ).then_inc(store_sem, 16)
                sync.wait_ge(s_dve, dve_total)
                sync.dma_start(
                    out=out_acc[:, 2 * m - 2 : 2 * m],
                    in_=acc[:, 2 * m - 2 : 2 * m],
                ).then_inc(store_sem, 16)
            else:
                sync.wait_ge(s_dve, dve_total)
                sync.dma_start(out=out_acc[:, :], in_=acc[:, :]).then_inc(
                    store_sem, 16
                )

        @block.scalar
        def _(scalar):
            if light_const_barrier:
                scalar.wait_ge(s_const, 1)
            # 1-col dummy Ln: pulls the ~1.3us ACT table load off the
            # first-tile critical path (it runs while p0 is in flight).
            dummy = scalar.activation(
                out=a_b[0][:, 0:1], in_=a_b[0][:, 0:1], func=Ln
            )
            if light_const_barrier:
                dummy.then_inc(s_const, 1)
            seen_tile = -1
            for c, (k, _, o, fs) in enumerate(chunks):
                if k != seen_tile:
                    p_ready(scalar, k)
                    seen_tile = k
                if c >= ab_bufs:
                    # a_b[c % ab_bufs] free once DVE consumed chunk c-ab_bufs
                    scalar.wait_ge(s_dve, dve_after[c - ab_bufs])
                scalar.activation(
                    out=a_b[c % ab_bufs][:, 0:fs],
                    in_=p_b[k][:, o : o + fs],
                    func=Ln,
                ).then_inc(s_act, 1)
                if c >= ab_bufs:
                    # b_b[c % ab_bufs] freed by its consumer
                    cc = c - ab_bufs
                    if cc in pool_set:
                        scalar.wait_ge(s_pool, pool_upto[cc])
                    else:
                        scalar.wait_ge(s_dve, dve_after[cc])
                bact = scalar.activation(
                    out=b_b[c % ab_bufs][:, 0:fs],
                    in_=p_b[k][:, o : o + fs],
                    func=Ln,
                    bias=1.0,
                    scale=-1.0,
                    accum_out=(
                        acc[:, bcol[c] : bcol[c] + 1] if c in pool_set else None
                    ),
                )
                bact.then_inc(s_act, 1)

        @block.vector
        def _(vector):
            seen_tile = -1
            for c, (k, _, o, fs) in enumerate(chunks):
                if k != seen_tile:
                    t_ready(vector, k)
                    seen_tile = k
                if c in pool_set:
                    # d = a-b ready once Pool's TT for c is done
                    vector.wait_ge(s_pool, pool_upto[c])
                    d_t = a_b[c % ab_bufs][:, 0:fs]
                    vector.scalar_tensor_tensor(
                        out=d_t,
                        in0=t_b[k][:, o : o + fs],
                        scalar=0.0,
                        in1=d_t,
                        op0=add,
                        op1=mult,
                        accum_out=acc[:, acol[c] : acol[c] + 1],
                    ).then_inc(s_dve, 1)
                else:
                    vector.wait_ge(s_act, 2 * c + 1)
                    a_t = a_b[c % ab_bufs][:, 0:fs]
                    vector.scalar_tensor_tensor(
                        out=a_t,
                        in0=t_b[k][:, o : o + fs],
                        scalar=0.0,
                        in1=a_t,
                        op0=add,
                        op1=mult,
                        accum_out=acc[:, acol[c] : acol[c] + 1],
                    ).then_inc(s_dve, 1)
                    vector.wait_ge(s_act, 2 * c + 2)
                    b_t = b_b[c % ab_bufs][:, 0:fs]
                    vector.scalar_tensor_tensor(
                        out=b_t,
                        in0=t_b[k][:, o : o + fs],
                        scalar=-1.0,
                        in1=b_t,
                        op0=add,
                        op1=mult,
                        accum_out=acc[:, bcol[c] : bcol[c] + 1],
                    ).then_inc(s_dve, 1)

        if pool_total:

            @block.gpsimd
            def _(gpsimd):
                for c, (k, _, o, fs) in enumerate(chunks):
                    if c not in pool_set:
                        continue
                    gpsimd.wait_ge(s_act, 2 * c + 2)
                    gpsimd.tensor_tensor(
                        out=a_b[c % ab_bufs][:, 0:fs],
                        in0=a_b[c % ab_bufs][:, 0:fs],
                        in1=b_b[c % ab_bufs][:, 0:fs],
                        op=mybir.AluOpType.subtract,
                    ).then_inc(s_pool, 1)

    import concourse.mybir as _mybir

    if drop_exit_barrier:
        # The Block-exit sem-only barrier (aeb_*) is redundant: walrus's
        # epilogue performs its own rendezvous right after.
        for blk in nc.m.functions[0].blocks:
            if blk.name == "main" or blk.name.endswith("_end"):
                blk.instructions = [
                    ins
                    for ins in blk.instructions
                    if not (
                        isinstance(ins, _mybir.InstEventSemaphore)
                        and getattr(ins, "name", "").startswith("aeb_")
                    )
                ]
    if strip_pe:
        # PE executes nothing; remove its drains so the kernel's BIR has
        # zero PE instructions.
        for blk in nc.m.functions[0].blocks:
            blk.instructions = [
                ins
                for ins in blk.instructions
                if getattr(ins, "engine", None) != _mybir.EngineType.PE
            ]
    nc.compile()
    nc._combine_signs_full = signs_full
    nc._combine_m = m
    return nc


N_CHUNKS = len(_chunks_of(DMA_TILES, CHUNK))


def _get_nc():
    if "nc" not in _CACHE:
        _CACHE["nc"] = _build()
    return _CACHE["nc"]


def bass_exec(preds, targets, nc=None):
    """Run the per-core Bass kernel on all 8 cores; returns results list."""
    _ensure_paths()
    from concourse.bass_utils import run_bass_kernel_spmd

    if nc is None:
        nc = _get_nc()
    in_maps = []
    for c in range(N_CORES):
        sl = slice(c * B_PER_CORE, (c + 1) * B_PER_CORE)
        in_maps.append(
            {
                "preds": np.ascontiguousarray(preds[sl]).reshape(P, FREE),
                "targets": np.ascontiguousarray(targets[sl]).reshape(P, FREE),
            }
        )
    return run_bass_kernel_spmd(nc, in_maps, core_ids=list(range(N_CORES)))


def _combine(results, signs_full=None):
    if signs_full is None:
        nc = _CACHE.get("nc")
        if nc is not None:
            signs_full = nc._combine_signs_full
        else:
            m = N_CHUNKS
            signs_full = np.concatenate(
                [np.ones(m - 1), -np.ones(m - 1), [1.0, -1.0]]
            )
    total = 0.0
    for core_out in results:
        a = np.asarray(core_out["acc"], dtype=np.float64)
        total += (a * signs_full).sum()
    return -total / N_TOTAL


def _count_components(mask):
    """Connected-component count, 4-connectivity (reference-equivalent)."""
    try:
        from scipy import ndimage

        return float(ndimage.label(mask)[1])
    except ImportError:
        pass
    return _count_components_np(mask)


def _count_components_np(mask):
    """Pure-numpy fallback: min-label propagation with pointer jumping."""
    Hm, Wm = mask.shape
    N = Hm * Wm
    idx = np.arange(N, dtype=np.int64).reshape(Hm, Wm)
    BIG = np.int64(N)
    lab = np.where(mask, idx, BIG)
    while True:
        up = np.concatenate([lab[1:], np.full((1, Wm), BIG, lab.dtype)], 0)
        down = np.concatenate([np.full((1, Wm), BIG, lab.dtype), lab[:-1]], 0)
        left = np.concatenate([lab[:, 1:], np.full((Hm, 1), BIG, lab.dtype)], 1)
        right = np.concatenate([np.full((Hm, 1), BIG, lab.dtype), lab[:, :-1]], 1)
        nm = np.minimum(np.minimum(up, down), np.minimum(left, right))
        new = np.where(mask, np.minimum(lab, nm), BIG)
        for _ in range(2):  # pointer jumping
            flat = new.reshape(-1)
            valid = flat < N
            safe = np.where(valid, flat, 0)
            flat = np.where(valid, flat[safe], BIG)
            new = flat.reshape(Hm, Wm)
        if np.array_equal(new, lab):
            break
        lab = new
    return float(np.sum(mask & (lab == idx)))


def kernel(preds, targets):
    preds = np.asarray(preds, dtype=np.float32)
    targets = np.asarray(targets, dtype=np.float32)
    assert preds.shape == (B, H, W) and targets.shape == (B, H, W)

    def _acc_ok(results):
        # every acc column is a sum of ~65k-262k nonzero products; an
        # exactly-zero column means a cold/wedged exec dropped it
        return all(
            np.all(np.any(np.asarray(core_out["acc"]), axis=0))
            for core_out in results
        )

    res = bass_exec(preds, targets)
    bce = _combine(res.results)
    if not np.isfinite(bce) or not _acc_ok(res.results):
        # a wedged/just-recovered device can return garbage once; one
        # clean re-execution flushes it
        res = bass_exec(preds, targets)
        bce = _combine(res.results)

    # connectivity penalty: 0 unless preds contains exact zeros
    if np.any(preds == 0.0):
        counts = [_count_components(preds[b] != 0.0) for b in range(B)]
        penalty = float(np.mean(np.asarray(counts) - 1.0))
    else:
        penalty = 0.0

    return np.float32(bce + penalty)
